# revision 43
# baseline (speedup 1.0000x reference)
"""GCNConv message-passing kernel for 8 Trainium2 NeuronCores.

Strategy (per spec sharding_hint: shard nodes, replicate theta):
  - Nodes are grouped into 128-node windows; windows are dealt round-robin to
    the 8 cores (the reference's edge generator concentrates src in the low
    node ids, so contiguous sharding would idle half the cores).  "Active"
    windows (those containing any edge source) are padded to a multiple of 8
    so every core owns exactly the same count.
  - Each core computes its shard of m = rsqrt(deg) * (x @ theta) on device
    (x passed pre-transposed so the matmul needs no device-side transpose),
    splits m into hi/lo bf16 halves (hi + lo == f32 m to ~1 ulp of bf16(lo)),
    and AllGathers the [12545, 128]-bf16 shard into a replicated table.
  - Edge messages are fetched with dma_gather (256B rows; int16 indices, so
    the table is addressed in 4 buckets of 2 shards each, one SWDGE queue per
    bucket, calls interleaved chunk-major so the 4 queues' drains overlap,
    batch-wide idx loads) and segment-reduced into the owning 128-node window
    with one-hot matmuls:
        psum[slot, ch] += sel^T @ msg,  sel[e, s] = (srcloc[e] == iota[s])
    built on the DVE via tensor_scalar is_equal (per-partition f32 scalar).
    Finally out[i] = norm[i] * (m[i] + agg[i]).
All loop structure is identical across cores; per-core variability lives in
host-packed index/srcloc data (segments padded to a common capacity S, pads
gather a zero table row and carry srcloc = -1 so they contribute nothing).
"""

import sys

sys.path.insert(0, "/opt/trn_rl_repo")

import numpy as np
import ml_dtypes

import concourse.bacc as bacc
import concourse.tile as tile
import concourse.mybir as mybir
from concourse import bass_utils, library_config

F32 = mybir.dt.float32
BF16 = mybir.dt.bfloat16
I16 = mybir.dt.int16
I32 = mybir.dt.int32
bf16 = ml_dtypes.bfloat16

N_NODES = 100000
IN_CH = 256
OUT_CH = 64
N_CORES = 8
P = 128
NW = 98                           # windows per core (98*128*8 = 100352 slots)
NPAD = NW * P                     # 12544 node slots per core
ROWS_K = NPAD + 1                 # 12545 table rows/shard (last = zero row)
TABLE_ROWS = N_CORES * ROWS_K
BUCKETS = 4                       # int16 gather idx < 32768 => 2 shards/bucket
BROWS = 2 * ROWS_K                # 25090 table rows per bucket
EW = 128                          # bf16 elems per table row (hi|lo) = 256B
KCH = IN_CH // P                  # 2 contraction halves
BW = 4                            # windows per batch (PSUM banks)
GMAX = 1024                       # dma_gather per-call index cap (SWDGE ring)
X_CHUNK = 1024                    # phase-1 xT streaming chunk (free dim)
GW_TOTAL = NW * N_CORES           # 784 global windows

_CACHE = {}


def _build(S, ACT_W, R1=1, R2=1, skip_gather=False, skip_compute=False,
           single_packet=True):
    """Build + compile the SPMD Bass program.

    S: padded segment capacity (multiple of 128); ACT_W: active windows/core.
    skip_gather/skip_compute: timing-dissection builds (output invalid).
    """
    SS = S // P
    NB = (ACT_W + BW - 1) // BW
    NWB = [min(BW, ACT_W - BW * i) for i in range(NB)]
    GCOLS = BUCKETS * ACT_W * S // 16
    LCOLS = BUCKETS * ACT_W * S // P
    nc = bacc.Bacc("TRN2", target_bir_lowering=False, debug=False,
                   num_devices=N_CORES, num_swdge_queues=4)
    xT = nc.dram_tensor("xT", [IN_CH, NPAD], F32, kind="ExternalInput")
    theta = nc.dram_tensor("theta", [IN_CH, OUT_CH], F32, kind="ExternalInput")
    deg = nc.dram_tensor("deg", [P, NW], F32, kind="ExternalInput")
    iota = nc.dram_tensor("iota", [P, P], BF16, kind="ExternalInput")
    gidx = nc.dram_tensor("gidx", [P, GCOLS], I16, kind="ExternalInput")
    srcloc = nc.dram_tensor("srcloc", [P, LCOLS], F32, kind="ExternalInput")
    out = nc.dram_tensor("out", [NPAD, OUT_CH], F32, kind="ExternalOutput")

    with tile.TileContext(nc) as tc:
        with (
            tc.tile_pool(name="persist", bufs=1) as pp,
            tc.tile_pool(name="dram", bufs=1, space="DRAM") as dp,
        ):
            nc.gpsimd.load_library(library_config.mlp)

            m_own = pp.tile([P, NW, OUT_CH], F32)
            out_sb = pp.tile([P, NW, OUT_CH], F32)
            norm = pp.tile([P, NW], F32)
            rec = pp.tile([P, NW], F32)
            degt = pp.tile([P, NW], F32)
            theta_sb = pp.tile([P, KCH, OUT_CH], F32)
            iota_sb = pp.tile([P, P], BF16)
            srcloc_sb = pp.tile([P, LCOLS], F32)
            stgs = [pp.tile([P, BW * BUCKETS * SS, EW], BF16, name=f"stg{i}")
                    for i in range(2)]
            m_k = dp.tile([ROWS_K, EW], BF16)

            nc.sync.dma_start(
                theta_sb[:], theta[:].rearrange("(k p) c -> p k c", p=P))
            nc.sync.dma_start(iota_sb[:], iota[:])
            nc.sync.dma_start(srcloc_sb[:], srcloc[:])
            nc.sync.dma_start(degt[:], deg[:])
            nc.vector.reciprocal(rec[:], degt[:])
            nc.scalar.activation(norm[:], rec[:],
                                 mybir.ActivationFunctionType.Sqrt)

            # ---- Phase 1: m = norm * (x @ theta); hi/lo bf16 split ----
            for _rep1 in range(R1):
              with (
                  tc.tile_pool(name="p1x", bufs=3) as p1x,
                  tc.tile_pool(name="p1big", bufs=1) as p1b,
                  tc.tile_pool(name="p1ps", bufs=4, space="PSUM") as p1ps,
              ):
                  mhilo = p1b.tile([P, NW, EW], BF16)
                  for c in range(0, NPAD, X_CHUNK):
                      cw = min(X_CHUNK, NPAD - c)
                      xa = p1x.tile([P, cw], F32, tag="xa")
                      xb = p1x.tile([P, cw], F32, tag="xb")
                      nc.sync.dma_start(xa[:], xT[0:P, c:c + cw])
                      nc.sync.dma_start(xb[:], xT[P:2 * P, c:c + cw])
                      for t in range(cw // P):
                          w = (c + t * P) // P
                          ph = p1ps.tile([P, OUT_CH], F32)
                          nc.tensor.matmul(ph[:], lhsT=xa[:, t * P:(t + 1) * P],
                                           rhs=theta_sb[:, 0, :],
                                           start=True, stop=False)
                          nc.tensor.matmul(ph[:], lhsT=xb[:, t * P:(t + 1) * P],
                                           rhs=theta_sb[:, 1, :],
                                           start=False, stop=True)
                          nc.scalar.activation(m_own[:, w, :], ph[:],
                                               mybir.ActivationFunctionType.Copy,
                                               scale=norm[:, w:w + 1])
                          hf = p1x.tile([P, OUT_CH], F32, tag="hf")
                          nc.vector.tensor_copy(mhilo[:, w, 0:OUT_CH],
                                                m_own[:, w, :])
                          nc.vector.tensor_copy(hf[:], mhilo[:, w, 0:OUT_CH])
                          nc.vector.tensor_sub(mhilo[:, w, OUT_CH:EW],
                                               m_own[:, w, :], hf[:])
                  # shard table -> DRAM (+ zero pad row), then AllGather
                  nc.sync.dma_start(
                      m_k[0:NPAD, :].rearrange("(w p) c -> p w c", p=P),
                      mhilo[:])
                  zrow = p1x.tile([1, EW], BF16, tag="z")
                  nc.vector.memset(zrow[:], 0)
                  nc.sync.dma_start(m_k[NPAD:ROWS_K, :], zrow[:])

                # Shared output allows only one writer inst; reps each get one.
              m_table = dp.tile([TABLE_ROWS, EW], BF16, addr_space="Shared",
                                name=f"m_table_r{_rep1}")
              nc.gpsimd.collective_compute(
                  "AllGather",
                  mybir.AluOpType.bypass,
                  replica_groups=[list(range(N_CORES))],
                  ins=[m_k.opt()],
                  outs=[m_table.opt()],
              )

            # ---- Phase 2: gather + one-hot matmul segment reduction ----
            for _rep2 in range(R2):
              with (
                  tc.tile_pool(name="idxp", bufs=6) as idxp,
                  tc.tile_pool(name="selp", bufs=24) as selp,
                  tc.tile_pool(name="epp", bufs=8) as epp,
                  tc.tile_pool(name="p2ps", bufs=8, space="PSUM") as p2ps,
              ):
                  gcol = 0
                  lofs = 0
                  for bi in range(NB):
                      nw = NWB[bi]
                      CB = nw * SS  # column-blocks per bucket this batch
                      stg = stgs[bi % 2]
                      L = nw * S
                      # one idx DMA per batch; chunk-major gather order so
                      # consecutive calls hit different SWDGE queues and
                      # their drains overlap.
                      bcols = BUCKETS * (L // 16)
                      it = idxp.tile([P, bcols], I16, tag="idx")
                      nc.sync.dma_start(it[:], gidx[:, gcol:gcol + bcols])
                      for c0 in range(0, L, GMAX):
                          Lc = min(GMAX, L - c0)
                          for b in range(BUCKETS):
                              i0 = b * (L // 16) + c0 // 16
                              d0 = b * CB + c0 // P
                              if not skip_gather:
                                  nc.gpsimd.dma_gather(
                                      stg[:, d0:d0 + Lc // P, :],
                                      m_table[b * BROWS:(b + 1) * BROWS, :],
                                      it[:, i0:i0 + Lc // 16],
                                      Lc, Lc, EW, queue_num=b,
                                      single_packet=single_packet)
                      gcol += bcols
                      for wi in range(nw if not skip_compute else 0):
                          w = bi * BW + wi
                          ps = p2ps.tile([P, OUT_CH], F32)
                          for b in range(BUCKETS):
                              for st in range(SS):
                                  col = b * CB + wi * SS + st
                                  g = lofs + col
                                  sel = selp.tile([P, P], BF16, tag="sel")
                                  nc.vector.tensor_scalar(
                                      sel[:], iota_sb[:],
                                      srcloc_sb[:, g:g + 1], None,
                                      op0=mybir.AluOpType.is_equal)
                                  first = (b == 0 and st == 0)
                                  last = (b == BUCKETS - 1 and st == SS - 1)
                                  nc.tensor.matmul(
                                      ps[:], lhsT=sel[:],
                                      rhs=stg[:, col, 0:OUT_CH],
                                      start=first, stop=False)
                                  nc.tensor.matmul(
                                      ps[:], lhsT=sel[:],
                                      rhs=stg[:, col, OUT_CH:EW],
                                      start=False, stop=last)
                          tmp = epp.tile([P, OUT_CH], F32, tag="ep")
                          nc.vector.tensor_add(tmp[:], ps[:], m_own[:, w, :])
                          nc.scalar.activation(out_sb[:, w, :], tmp[:],
                                               mybir.ActivationFunctionType.Copy,
                                               scale=norm[:, w:w + 1])
                      lofs += BUCKETS * CB
                  # passive windows: agg == 0 -> out = norm * m
                  for w in range(ACT_W, NW):
                      nc.scalar.activation(out_sb[:, w, :], m_own[:, w, :],
                                           mybir.ActivationFunctionType.Copy,
                                           scale=norm[:, w:w + 1])

            nc.sync.dma_start(
                out[:].rearrange("(w p) c -> p w c", p=P), out_sb[:])
    nc.compile()
    return nc


def _node_maps(act_gw):
    """Global window -> (core, local window); active windows round-robin."""
    gw = np.arange(GW_TOTAL)
    core_of_gw = np.where(gw < act_gw, gw % N_CORES, (gw - act_gw) % N_CORES)
    lw_of_gw = np.where(gw < act_gw, gw // N_CORES,
                        act_gw // N_CORES + (gw - act_gw) // N_CORES)
    return core_of_gw, lw_of_gw


def _prepare(x, theta, edge_index):
    """Host-side sharding: per-core input dicts + structure params."""
    src = np.asarray(edge_index[0], dtype=np.int64)
    dst = np.asarray(edge_index[1], dtype=np.int64)
    E = src.shape[0]

    deg = 1.0 + np.bincount(src, minlength=N_NODES).astype(np.float64)

    # active windows = those that may contain an edge source
    act_gw = -(-int(src.max() + 1) // P)
    act_gw = min(-(-act_gw // N_CORES) * N_CORES, GW_TOTAL)
    ACT_W = act_gw // N_CORES

    core_of_gw, lw_of_gw = _node_maps(act_gw)

    sgw = src >> 7
    core = core_of_gw[sgw]
    win = lw_of_gw[sgw]                    # < ACT_W by construction
    slot = src & (P - 1)
    dgw = dst >> 7
    dcore = core_of_gw[dgw]
    dl = lw_of_gw[dgw] * P + (dst & (P - 1))
    bucket = dcore // 2
    blocal = (dcore % 2) * ROWS_K + dl     # gather idx within bucket

    batch = win // BW
    order = np.lexsort((blocal, win, bucket, batch, core))

    seg_key = (core * ACT_W + win) * BUCKETS + bucket
    counts = np.bincount(seg_key, minlength=N_CORES * ACT_W * BUCKETS)
    S = int(-(-max(1, int(counts.max())) // P) * P)

    NB = (ACT_W + BW - 1) // BW
    NWB = np.array([min(BW, ACT_W - BW * i) for i in range(NB)])
    core_sz = BUCKETS * ACT_W * S
    batch_base = np.concatenate([[0], np.cumsum(NWB * BUCKETS * S)])[:-1]
    seg_off = (core * core_sz + batch_base[batch]
               + bucket * (NWB[batch] * S) + (win - batch * BW) * S)

    ks = seg_key[order]
    new = np.empty(E, dtype=bool)
    new[0] = True
    np.not_equal(ks[1:], ks[:-1], out=new[1:])
    gstart = np.flatnonzero(new)
    rank = np.arange(E) - np.repeat(gstart, np.diff(np.r_[gstart, E]))
    pos = seg_off[order] + rank

    total = N_CORES * core_sz
    gidx_flat = np.full(total, ROWS_K - 1, dtype=np.int16)  # pad -> zero row
    gidx_flat[pos] = blocal[order].astype(np.int16)
    srcloc_flat = np.full(total, -1.0, dtype=np.float32)
    srcloc_flat[pos] = slot[order].astype(np.float32)

    # inverse node map: per-core slot -> global node (or -1)
    inv = np.full(N_CORES * NPAD, -1, dtype=np.int64)
    g = np.arange(N_NODES)
    gcore = core_of_gw[g >> 7]
    glocal = lw_of_gw[g >> 7] * P + (g & (P - 1))
    inv[gcore * NPAD + glocal] = g

    iota_np = np.broadcast_to(
        np.arange(P, dtype=np.float32), (P, P)).astype(bf16).copy()
    theta_np = np.ascontiguousarray(np.asarray(theta, dtype=np.float32))

    x = np.asarray(x, dtype=np.float32)
    in_maps = []
    for k in range(N_CORES):
        invk = inv[k * NPAD:(k + 1) * NPAD]
        real = invk >= 0
        xk = np.zeros((NPAD, IN_CH), dtype=np.float32)
        xk[real] = x[invk[real]]
        xkT = np.ascontiguousarray(xk.T)
        dg = np.ones(NPAD, dtype=np.float32)
        dg[real] = deg[invk[real]]
        dg = np.ascontiguousarray(dg.reshape(NW, P).T)

        cflat = gidx_flat[k * core_sz:(k + 1) * core_sz]
        blocks = []
        off = 0
        for bi in range(NB):
            L = int(NWB[bi]) * S
            for b in range(BUCKETS):
                for c0 in range(0, L, GMAX):
                    Lc = min(GMAX, L - c0)
                    blocks.append(
                        cflat[off:off + Lc].reshape(Lc // 16, 16).T)
                    off += Lc
        g16 = np.concatenate(blocks, axis=1)
        gpack = np.ascontiguousarray(np.tile(g16, (8, 1)))

        lflat = srcloc_flat[k * core_sz:(k + 1) * core_sz]
        lpack = np.ascontiguousarray(
            lflat.reshape(core_sz // P, P).T)

        in_maps.append({
            "xT": xkT,
            "theta": theta_np,
            "deg": dg,
            "iota": iota_np,
            "gidx": gpack,
            "srcloc": lpack,
        })
    meta = (S, ACT_W, gcore, glocal)
    return in_maps, meta


def kernel(x, theta, edge_index):
    in_maps, (S, ACT_W, gcore, glocal) = _prepare(x, theta, edge_index)
    key = (S, ACT_W)
    if key not in _CACHE:
        _CACHE[key] = _build(S, ACT_W)
    nc = _CACHE[key]
    res = bass_utils.run_bass_kernel_spmd(
        nc, in_maps, core_ids=list(range(N_CORES)))
    allp = np.stack([res.results[k]["out"] for k in range(N_CORES)], axis=0)
    return np.ascontiguousarray(allp[gcore, glocal])



# revision 58
# speedup vs baseline: 1.7524x; 1.7524x over previous
"""GCNConv message-passing kernel for 8 Trainium2 NeuronCores.

Strategy (per spec sharding_hint: shard nodes, replicate theta):
  - Nodes are grouped into 128-node windows; windows are dealt round-robin to
    the 8 cores (the reference's edge generator concentrates src in the low
    node ids, so contiguous sharding would idle half the cores).  "Active"
    windows (those containing any edge source) are padded to a multiple of 8
    so every core owns exactly the same count.
  - Each core computes its shard of m = rsqrt(deg) * (x @ theta) on device
    (x passed pre-transposed so the matmul needs no device-side transpose),
    splits m into hi/lo bf16 halves (hi + lo == f32 m to ~1 ulp of bf16(lo)),
    and AllGathers the [12545, 128]-bf16 shard into a replicated table.
  - Edge messages are fetched with dma_gather (256B rows; int16 indices, so
    the table is addressed in 4 buckets of 2 shards each, one SWDGE queue per
    bucket, calls interleaved chunk-major so the 4 queues' drains overlap,
    batch-wide idx loads) and segment-reduced into the owning 128-node window
    with one-hot matmuls:
        psum[slot, ch] += sel^T @ msg,  sel[e, s] = (srcloc[e] == iota[s])
    built on the DVE via tensor_scalar is_equal (per-partition f32 scalar).
    Finally out[i] = norm[i] * (m[i] + agg[i]).
All loop structure is identical across cores; per-core variability lives in
host-packed index/srcloc data (segments padded to a common capacity S, pads
gather a zero table row and carry srcloc = -1 so they contribute nothing).
"""

import sys

sys.path.insert(0, "/opt/trn_rl_repo")

import numpy as np
import ml_dtypes

import concourse.bacc as bacc
import concourse.tile as tile
import concourse.mybir as mybir
from concourse import bass_utils, library_config

F32 = mybir.dt.float32
BF16 = mybir.dt.bfloat16
I16 = mybir.dt.int16
I32 = mybir.dt.int32
bf16 = ml_dtypes.bfloat16

N_NODES = 100000
IN_CH = 256
OUT_CH = 64
N_CORES = 8
P = 128
NW = 98                           # windows per core (98*128*8 = 100352 slots)
NPAD = NW * P                     # 12544 node slots per core
ROWS_K = NPAD + 1                 # 12545 table rows/shard (last = zero row)
TABLE_ROWS = N_CORES * ROWS_K
BUCKETS = 4                       # int16 gather idx < 32768 => 2 shards/bucket
BROWS = 2 * ROWS_K                # 25090 table rows per bucket
EW = 128                          # bf16 elems per table row (hi|lo) = 256B
KCH = IN_CH // P                  # 2 contraction halves
BW = 4                            # windows per batch (PSUM banks)
GMAX = 1024                       # dma_gather per-call index cap (SWDGE ring)
X_CHUNK = 1024                    # phase-1 xT streaming chunk (free dim)
GW_TOTAL = NW * N_CORES           # 784 global windows

_CACHE = {}


def _build(S, ACT_W, R1=1, R2=1, skip_gather=False, skip_compute=False,
           single_packet=True):
    """Build + compile the SPMD Bass program.

    S: padded segment capacity (multiple of 128); ACT_W: active windows/core.
    skip_gather/skip_compute: timing-dissection builds (output invalid).
    """
    SS = S // P
    NB = (ACT_W + BW - 1) // BW
    NWB = [min(BW, ACT_W - BW * i) for i in range(NB)]
    GCOLS = BUCKETS * ACT_W * S // 16
    LCOLS = BUCKETS * ACT_W * S // P
    nc = bacc.Bacc("TRN2", target_bir_lowering=False, debug=False,
                   num_devices=N_CORES, num_swdge_queues=4)
    xT = nc.dram_tensor("xT", [IN_CH, NPAD], F32, kind="ExternalInput")
    theta = nc.dram_tensor("theta", [IN_CH, OUT_CH], F32, kind="ExternalInput")
    deg = nc.dram_tensor("deg", [P, NW], F32, kind="ExternalInput")
    iota = nc.dram_tensor("iota", [P, SS * P], BF16, kind="ExternalInput")
    gidx = nc.dram_tensor("gidx", [P, GCOLS], I16, kind="ExternalInput")
    srcloc = nc.dram_tensor("srcloc", [P, LCOLS], F32, kind="ExternalInput")
    out = nc.dram_tensor("out", [NPAD, OUT_CH], F32, kind="ExternalOutput")

    with tile.TileContext(nc) as tc:
        with (
            tc.tile_pool(name="persist", bufs=1) as pp,
            tc.tile_pool(name="dram", bufs=1, space="DRAM") as dp,
        ):
            nc.gpsimd.load_library(library_config.mlp)

            m_own = pp.tile([P, NW, OUT_CH], F32)
            out_sb = pp.tile([P, NW, OUT_CH], F32)
            norm = pp.tile([P, NW], F32)
            rec = pp.tile([P, NW], F32)
            degt = pp.tile([P, NW], F32)
            theta_sb = pp.tile([P, KCH, OUT_CH], F32)
            iota_sb = pp.tile([P, SS * P], BF16)
            srcloc_sb = pp.tile([P, LCOLS], F32)
            stgs = [pp.tile([P, BW * BUCKETS * SS, EW], BF16, name=f"stg{i}")
                    for i in range(2)]
            m_k = dp.tile([ROWS_K, EW], BF16)

            nc.sync.dma_start(
                theta_sb[:], theta[:].rearrange("(k p) c -> p k c", p=P))
            nc.sync.dma_start(iota_sb[:], iota[:])
            nc.sync.dma_start(srcloc_sb[:], srcloc[:])
            nc.sync.dma_start(degt[:], deg[:])
            nc.vector.reciprocal(rec[:], degt[:])
            nc.scalar.activation(norm[:], rec[:],
                                 mybir.ActivationFunctionType.Sqrt)

            # ---- Phase 1: m = norm * (x @ theta); hi/lo bf16 split ----
            for _rep1 in range(R1):
              with (
                  tc.tile_pool(name="p1x", bufs=3) as p1x,
                  tc.tile_pool(name="p1big", bufs=1) as p1b,
                  tc.tile_pool(name="p1ps", bufs=4, space="PSUM") as p1ps,
              ):
                  mhilo = p1b.tile([P, NW, EW], BF16)
                  for c in range(0, NPAD, X_CHUNK):
                      cw = min(X_CHUNK, NPAD - c)
                      xa = p1x.tile([P, cw], F32, tag="xa")
                      xb = p1x.tile([P, cw], F32, tag="xb")
                      nc.sync.dma_start(xa[:], xT[0:P, c:c + cw])
                      nc.sync.dma_start(xb[:], xT[P:2 * P, c:c + cw])
                      for t in range(cw // P):
                          w = (c + t * P) // P
                          ph = p1ps.tile([P, OUT_CH], F32)
                          nc.tensor.matmul(ph[:], lhsT=xa[:, t * P:(t + 1) * P],
                                           rhs=theta_sb[:, 0, :],
                                           start=True, stop=False)
                          nc.tensor.matmul(ph[:], lhsT=xb[:, t * P:(t + 1) * P],
                                           rhs=theta_sb[:, 1, :],
                                           start=False, stop=True)
                          nc.scalar.activation(m_own[:, w, :], ph[:],
                                               mybir.ActivationFunctionType.Copy,
                                               scale=norm[:, w:w + 1])
                          hf = p1x.tile([P, OUT_CH], F32, tag="hf")
                          nc.vector.tensor_copy(mhilo[:, w, 0:OUT_CH],
                                                m_own[:, w, :])
                          nc.vector.tensor_copy(hf[:], mhilo[:, w, 0:OUT_CH])
                          nc.vector.tensor_sub(mhilo[:, w, OUT_CH:EW],
                                               m_own[:, w, :], hf[:])
                  # shard table -> DRAM (+ zero pad row), then AllGather
                  nc.sync.dma_start(
                      m_k[0:NPAD, :].rearrange("(w p) c -> p w c", p=P),
                      mhilo[:])
                  zrow = p1x.tile([1, EW], BF16, tag="z")
                  nc.vector.memset(zrow[:], 0)
                  nc.sync.dma_start(m_k[NPAD:ROWS_K, :], zrow[:])

                # Shared output allows only one writer inst; reps each get one.
              m_table = dp.tile([TABLE_ROWS, EW], BF16, addr_space="Shared",
                                name=f"m_table_r{_rep1}")
              nc.gpsimd.collective_compute(
                  "AllGather",
                  mybir.AluOpType.bypass,
                  replica_groups=[list(range(N_CORES))],
                  ins=[m_k.opt()],
                  outs=[m_table.opt()],
              )

            # ---- Phase 2: gather + one-hot matmul segment reduction ----
            for _rep2 in range(R2):
              with (
                  tc.tile_pool(name="idxp", bufs=6) as idxp,
                  tc.tile_pool(name="selp", bufs=4) as selp,
                  tc.tile_pool(name="epp", bufs=8) as epp,
                  tc.tile_pool(name="p2ps", bufs=8, space="PSUM") as p2ps,
              ):
                  gcol = 0
                  lofs = 0
                  for bi in range(NB):
                      nw = NWB[bi]
                      CB = nw * SS  # column-blocks per bucket this batch
                      stg = stgs[bi % 2]
                      L = nw * S
                      # one idx DMA per batch; chunk-major gather order so
                      # consecutive calls hit different SWDGE queues and
                      # their drains overlap.
                      bcols = BUCKETS * (L // 16)
                      it = idxp.tile([P, bcols], I16, tag="idx")
                      nc.sync.dma_start(it[:], gidx[:, gcol:gcol + bcols])
                      for c0 in range(0, L, GMAX):
                          Lc = min(GMAX, L - c0)
                          for b in range(BUCKETS):
                              i0 = b * (L // 16) + c0 // 16
                              d0 = b * CB + c0 // P
                              if not skip_gather:
                                  nc.gpsimd.dma_gather(
                                      stg[:, d0:d0 + Lc // P, :],
                                      m_table[b * BROWS:(b + 1) * BROWS, :],
                                      it[:, i0:i0 + Lc // 16],
                                      Lc, Lc, EW, queue_num=b,
                                      single_packet=single_packet)
                      gcol += bcols
                      for wi in range(nw if not skip_compute else 0):
                          w = bi * BW + wi
                          ps = p2ps.tile([P, OUT_CH], F32)
                          for b in range(BUCKETS):
                              g0 = lofs + b * CB + wi * SS
                              # batch the SS subtile one-hots: broadcast-copy
                              # srcloc then one unit-stride bf16 is_equal
                              # (2x DVE mode) against the tiled iota.
                              rep = selp.tile([P, SS, P], BF16, tag="rep",
                                              bufs=3)
                              nc.vector.tensor_copy(
                                  rep[:],
                                  srcloc_sb[:, g0:g0 + SS]
                                  .to_broadcast([P, SS, P]))
                              sel9 = selp.tile([P, SS, P], BF16, tag="sel")
                              nc.vector.tensor_tensor(
                                  sel9[:], rep[:],
                                  iota_sb[:].rearrange("p (s q) -> p s q",
                                                       q=P),
                                  op=mybir.AluOpType.is_equal)
                              for st in range(SS):
                                  col = b * CB + wi * SS + st
                                  first = (b == 0 and st == 0)
                                  last = (b == BUCKETS - 1 and st == SS - 1)
                                  nc.tensor.matmul(
                                      ps[:], lhsT=sel9[:, st, :],
                                      rhs=stg[:, col, 0:OUT_CH],
                                      start=first, stop=False)
                                  nc.tensor.matmul(
                                      ps[:], lhsT=sel9[:, st, :],
                                      rhs=stg[:, col, OUT_CH:EW],
                                      start=False, stop=last)
                          tmp = epp.tile([P, OUT_CH], F32, tag="ep")
                          nc.vector.tensor_add(tmp[:], ps[:], m_own[:, w, :])
                          nc.scalar.activation(out_sb[:, w, :], tmp[:],
                                               mybir.ActivationFunctionType.Copy,
                                               scale=norm[:, w:w + 1])
                      lofs += BUCKETS * CB
                  # passive windows: agg == 0 -> out = norm * m
                  for w in range(ACT_W, NW):
                      nc.scalar.activation(out_sb[:, w, :], m_own[:, w, :],
                                           mybir.ActivationFunctionType.Copy,
                                           scale=norm[:, w:w + 1])

            nc.sync.dma_start(
                out[:].rearrange("(w p) c -> p w c", p=P), out_sb[:])
    nc.compile()
    return nc


def _node_maps(act_gw):
    """Global window -> (core, local window); active windows round-robin."""
    gw = np.arange(GW_TOTAL)
    core_of_gw = np.where(gw < act_gw, gw % N_CORES, (gw - act_gw) % N_CORES)
    lw_of_gw = np.where(gw < act_gw, gw // N_CORES,
                        act_gw // N_CORES + (gw - act_gw) // N_CORES)
    return core_of_gw, lw_of_gw


def _prepare(x, theta, edge_index):
    """Host-side sharding: per-core input dicts + structure params."""
    src = np.asarray(edge_index[0], dtype=np.int64)
    dst = np.asarray(edge_index[1], dtype=np.int64)
    E = src.shape[0]

    deg = 1.0 + np.bincount(src, minlength=N_NODES).astype(np.float64)

    # active windows = those that may contain an edge source
    act_gw = -(-int(src.max() + 1) // P)
    act_gw = min(-(-act_gw // N_CORES) * N_CORES, GW_TOTAL)
    ACT_W = act_gw // N_CORES

    core_of_gw, lw_of_gw = _node_maps(act_gw)

    sgw = src >> 7
    core = core_of_gw[sgw]
    win = lw_of_gw[sgw]                    # < ACT_W by construction
    slot = src & (P - 1)
    dgw = dst >> 7
    dcore = core_of_gw[dgw]
    dl = lw_of_gw[dgw] * P + (dst & (P - 1))
    bucket = dcore // 2
    blocal = (dcore % 2) * ROWS_K + dl     # gather idx within bucket

    batch = win // BW
    order = np.lexsort((blocal, win, bucket, batch, core))

    seg_key = (core * ACT_W + win) * BUCKETS + bucket
    counts = np.bincount(seg_key, minlength=N_CORES * ACT_W * BUCKETS)
    S = int(-(-max(1, int(counts.max())) // P) * P)

    NB = (ACT_W + BW - 1) // BW
    NWB = np.array([min(BW, ACT_W - BW * i) for i in range(NB)])
    core_sz = BUCKETS * ACT_W * S
    batch_base = np.concatenate([[0], np.cumsum(NWB * BUCKETS * S)])[:-1]
    seg_off = (core * core_sz + batch_base[batch]
               + bucket * (NWB[batch] * S) + (win - batch * BW) * S)

    ks = seg_key[order]
    new = np.empty(E, dtype=bool)
    new[0] = True
    np.not_equal(ks[1:], ks[:-1], out=new[1:])
    gstart = np.flatnonzero(new)
    rank = np.arange(E) - np.repeat(gstart, np.diff(np.r_[gstart, E]))
    pos = seg_off[order] + rank

    total = N_CORES * core_sz
    gidx_flat = np.full(total, ROWS_K - 1, dtype=np.int16)  # pad -> zero row
    gidx_flat[pos] = blocal[order].astype(np.int16)
    srcloc_flat = np.full(total, -1.0, dtype=np.float32)
    srcloc_flat[pos] = slot[order].astype(np.float32)

    # inverse node map: per-core slot -> global node (or -1)
    inv = np.full(N_CORES * NPAD, -1, dtype=np.int64)
    g = np.arange(N_NODES)
    gcore = core_of_gw[g >> 7]
    glocal = lw_of_gw[g >> 7] * P + (g & (P - 1))
    inv[gcore * NPAD + glocal] = g

    SS = S // P
    iota_np = np.ascontiguousarray(np.tile(
        np.arange(P, dtype=np.float32).astype(bf16)[None, :], (P, SS)))
    theta_np = np.ascontiguousarray(np.asarray(theta, dtype=np.float32))

    x = np.asarray(x, dtype=np.float32)
    in_maps = []
    for k in range(N_CORES):
        invk = inv[k * NPAD:(k + 1) * NPAD]
        real = invk >= 0
        xk = np.zeros((NPAD, IN_CH), dtype=np.float32)
        xk[real] = x[invk[real]]
        xkT = np.ascontiguousarray(xk.T)
        dg = np.ones(NPAD, dtype=np.float32)
        dg[real] = deg[invk[real]]
        dg = np.ascontiguousarray(dg.reshape(NW, P).T)

        cflat = gidx_flat[k * core_sz:(k + 1) * core_sz]
        blocks = []
        off = 0
        for bi in range(NB):
            L = int(NWB[bi]) * S
            for b in range(BUCKETS):
                for c0 in range(0, L, GMAX):
                    Lc = min(GMAX, L - c0)
                    blocks.append(
                        cflat[off:off + Lc].reshape(Lc // 16, 16).T)
                    off += Lc
        g16 = np.concatenate(blocks, axis=1)
        gpack = np.ascontiguousarray(np.tile(g16, (8, 1)))

        lflat = srcloc_flat[k * core_sz:(k + 1) * core_sz]
        lpack = np.ascontiguousarray(
            lflat.reshape(core_sz // P, P).T)

        in_maps.append({
            "xT": xkT,
            "theta": theta_np,
            "deg": dg,
            "iota": iota_np,
            "gidx": gpack,
            "srcloc": lpack,
        })
    meta = (S, ACT_W, gcore, glocal)
    return in_maps, meta


def kernel(x, theta, edge_index):
    in_maps, (S, ACT_W, gcore, glocal) = _prepare(x, theta, edge_index)
    key = (S, ACT_W)
    if key not in _CACHE:
        _CACHE[key] = _build(S, ACT_W)
    nc = _CACHE[key]
    res = bass_utils.run_bass_kernel_spmd(
        nc, in_maps, core_ids=list(range(N_CORES)))
    allp = np.stack([res.results[k]["out"] for k in range(N_CORES)], axis=0)
    return np.ascontiguousarray(allp[gcore, glocal])

